# revision 48
# baseline (speedup 1.0000x reference)
"""GPTQ group-quantized linear (nn_GPTQLinear) on 8 Trainium2 NeuronCores.

out[b,s,o] = sum_k x[b,s,k] * (qweight[o,k] * scales[o, k//128]) + bias[o]

Full inputs in, full output out.  Sharding (internal): 4-way over batch rows
x 2-way over out_features -> per core M=2048 rows, N=2048 out feats, K=4096.

Per-core kernel (v2 -- no on-device transposes at all):
  - qweight is marshaled on the host into a [K, N] int32 shard, so the
    device loads [128k, 512o] slabs via SWDGE cast-DMA straight to bf16
    (values 0..15 are exact) already in the k-major matmul layout.
  - scales are marshaled [KT, N]; per (group, n-chunk) a K=1 ones-matmul
    broadcasts the scale row to 128 partitions in PSUM, and the DVE dequant
    multiply reads that PSUM tile directly: wT = q_bf16 * scale_bc.
  - x is marshaled [K, M] bf16; xT k-slices stream with plain strided DMAs
    split across both HWDGE rings.
  - bf16 matmuls (lhsT = xT slice, rhs = wT slice), fp32 accumulate in PSUM
    over K; bias added during the PSUM->SBUF drain.  The first two
    super-chunks use per-(sc,n) output writes so their matmuls interleave
    with the dequant stream; later super-chunks drain into [128, N] tiles
    flushed as single large writes.
"""

from contextlib import ExitStack

import numpy as np

import concourse.bass as bass
import concourse.bacc as bacc
import concourse.mybir as mybir
import concourse.tile as tile
from concourse import bass_utils

F32 = mybir.dt.float32
BF16 = mybir.dt.bfloat16
I32 = mybir.dt.int32

P = 128            # partitions = k-tile = quant group size
N_CH = 512         # out-feature chunk (one PSUM bank of fp32)
M_SC = 256         # x rows per super-chunk

# full problem / sharding constants (hardcoded per harness contract)
B, S, K_FULL, NF = 4, 2048, 4096, 4096
MB_SHARDS, NB_SHARDS = 4, 2
M_CORE, N_CORE = (B * S) // MB_SHARDS, NF // NB_SHARDS
N_CORES = 8


def emit(tc, ctx, o_ap, x_ap, q_ap, s_ap, b_ap):
    nc = tc.nc
    K, M = x_ap.shape   # x arrives pre-transposed [K, M] bf16 from the host
    N = q_ap.shape[1]   # qweight arrives pre-transposed [K, N] int32
    KT = K // P
    NCH = N // N_CH
    NSC = M // M_SC
    MT = M_SC // P

    const = ctx.enter_context(tc.tile_pool(name="const", bufs=1))
    wt_pool = ctx.enter_context(tc.tile_pool(name="wt", bufs=1))
    qs_pool = ctx.enter_context(tc.tile_pool(name="qs", bufs=4))
    xt_pool = ctx.enter_context(tc.tile_pool(name="xt", bufs=2))
    out_pool = ctx.enter_context(tc.tile_pool(name="outp", bufs=2))
    outs_pool = ctx.enter_context(tc.tile_pool(name="outs", bufs=2))
    pssc_pool = ctx.enter_context(tc.tile_pool(name="pssc", bufs=2, space="PSUM"))
    psmm_pool = ctx.enter_context(tc.tile_pool(name="psmm", bufs=5, space="PSUM"))

    # ---- constants ----
    srow_pool = ctx.enter_context(tc.tile_pool(name="srow", bufs=4))
    bias_sb = const.tile([1, N], BF16, tag="bias")
    nc.gpsimd.dma_start(bias_sb[:], b_ap[None, :])
    ones = const.tile([1, P], BF16, tag="ones")
    nc.vector.memset(ones[:], 1.0)

    # bias broadcast to all 128 partitions via a K=1 matmul
    bias_bc = const.tile([P, N], BF16, tag="bias_bc")
    for n in range(NCH):
        psb = psmm_pool.tile([P, N_CH], F32, bufs=1, name="psb")
        nc.tensor.matmul(
            psb[:], ones[:], bias_sb[:, n * N_CH : (n + 1) * N_CH],
            start=True, stop=True,
        )
        nc.vector.tensor_copy(bias_bc[:, n * N_CH : (n + 1) * N_CH], psb[:])

    wt = [
        wt_pool.tile([P, KT, N_CH], BF16, tag=f"wt{n}", name=f"wt{n}")
        for n in range(NCH)
    ]

    def dequant_group(n):
        """Dequantize the [K, 512] column block n of qT into wt[n]."""
        for kt in range(KT):
            # plain int32 loads split across BOTH HWDGE rings: the dequant
            # stream arrives ~2x faster than one serial SWDGE cast queue,
            # and the DVE multiply casts i32 -> bf16 on the fly.
            qt = qs_pool.tile([P, N_CH], I32, name="qt")
            eng = nc.sync if kt % 2 == 0 else nc.scalar
            eng.dma_start(
                qt[:],
                q_ap[kt * P : (kt + 1) * P, n * N_CH : (n + 1) * N_CH],
            )
            # scale row for group kt, broadcast to 128 partitions in PSUM
            srow = srow_pool.tile([1, N_CH], BF16, name="srow")
            nc.scalar.dma_start(
                srow[:], s_ap[kt : kt + 1, n * N_CH : (n + 1) * N_CH]
            )
            sbc = pssc_pool.tile([P, N_CH], F32, name="sbc")
            nc.tensor.matmul(sbc[:], ones[:], srow[:], start=True, stop=True)
            nc.vector.tensor_tensor(
                wt[n][:, kt, :], qt[:], sbc[:], mybir.AluOpType.mult
            )

    def load_xt(sc):
        # x is already [K, M] bf16 in DRAM: plain strided loads, split across
        # both HWDGE rings.  Per-kt tiles: matmuls depend only on the
        # k-slices they read, so accumulations start as slices land.
        xt = []
        for kt in range(KT):
            t = xt_pool.tile([P, M_SC], BF16, tag=f"xt{kt}", name=f"xt{kt}")
            eng = nc.sync if kt % 2 == 0 else nc.scalar
            eng.dma_start(
                t[:],
                x_ap[kt * P : (kt + 1) * P, sc * M_SC : (sc + 1) * M_SC],
            )
            xt.append(t)
        return xt

    def alloc_obig():
        # one [P, N] fp32 accumulation tile per m-tile of the super-chunk;
        # flushed as a single large contiguous write per m-tile.
        return [
            out_pool.tile([P, N], F32, tag=f"ot{mt}", name=f"ot{mt}", bufs=1)
            for mt in range(MT)
        ]

    def mm_core(xt, mt, n):
        ps = psmm_pool.tile([P, N_CH], F32, name="ps_mm")
        for kt in range(KT):
            nc.tensor.matmul(
                ps[:],
                xt[kt][:, mt * P : (mt + 1) * P],
                wt[n][:, kt, :],
                start=(kt == 0),
                stop=(kt == KT - 1),
            )
        return ps

    def mm_block(xt, sc, n, obig):
        for mt in range(MT):
            ps = mm_core(xt, mt, n)
            nc.vector.tensor_tensor(
                obig[mt][:, n * N_CH : (n + 1) * N_CH],
                ps[:],
                bias_bc[:, n * N_CH : (n + 1) * N_CH],
                mybir.AluOpType.add,
            )

    def mm_block_small(xt, sc, n):
        # phase-A variant: per-(sc, n) output writes so two super-chunks can
        # interleave with the dequant stream without holding obig tiles.
        for mt in range(MT):
            ps = mm_core(xt, mt, n)
            ot = outs_pool.tile([P, N_CH], F32, name="ot_s")
            nc.vector.tensor_tensor(
                ot[:], ps[:], bias_bc[:, n * N_CH : (n + 1) * N_CH],
                mybir.AluOpType.add,
            )
            m0 = sc * M_SC + mt * P
            nc.gpsimd.dma_start(
                o_ap[m0 : m0 + P, n * N_CH : (n + 1) * N_CH], ot[:]
            )

    def flush_out(sc, obig):
        for mt in range(MT):
            m0 = sc * M_SC + mt * P
            nc.gpsimd.dma_start(o_ap[m0 : m0 + P, :], obig[mt][:])

    # ---- interleave: dequant chunk g, then matmuls of (sc0/sc1, n=g), so
    # the PE has two super-chunks of matmul work per dequantized chunk and
    # stays busy while the qweight stream is still arriving.
    # one-group deferral: group g's dequant multiplies are queued on the DVE
    # before group g-1's psum-drain ADDs, so a drain waiting on its matmul
    # run never head-blocks the dequant stream.
    dequant_group(0)
    xt0 = load_xt(0)
    xt1 = load_xt(1)
    for g in range(1, NCH):
        dequant_group(g)
        mm_block_small(xt0, 0, g - 1)
        mm_block_small(xt1, 1, g - 1)
    mm_block_small(xt0, 0, NCH - 1)
    mm_block_small(xt1, 1, NCH - 1)
    for sc in range(2, NSC):
        xt = load_xt(sc)
        ob = alloc_obig()
        for n in range(NCH):
            mm_block(xt, sc, n, ob)
        flush_out(sc, ob)


def build_program(M=M_CORE, N=N_CORE, K=K_FULL):
    nc = bacc.Bacc("TRN2", target_bir_lowering=False, debug=False)
    x = nc.dram_tensor("x", [K, M], BF16, kind="ExternalInput")
    q = nc.dram_tensor("qweight", [K, N], I32, kind="ExternalInput")
    s = nc.dram_tensor("scales", [K // P, N], BF16, kind="ExternalInput")
    b = nc.dram_tensor("bias", [N], F32, kind="ExternalInput")
    o = nc.dram_tensor("out", [M, N], F32, kind="ExternalOutput")
    with tile.TileContext(nc) as tc:
        with ExitStack() as ctx:
            emit(tc, ctx, o.ap(), x.ap(), q.ap(), s.ap(), b.ap())
    nc.compile()
    return nc


def enable_ntff_profiling():
    """Register the axon NTFF profile hook (the image's antenv lacks
    axon_hooks, so trn_boot degrades silently).  Returns True on success."""
    import sys
    import types

    try:
        from antenv.axon_hooks import get_axon_ntff_profile_hook  # noqa: F401

        return True
    except ImportError:
        pass
    try:
        from trn_agent_boot.trn_boot import _ntff_profile_via_ctypes

        hook = _ntff_profile_via_ctypes("/opt/axon/libaxon_pjrt.so")
        if hook is None:
            return False
        mod = types.ModuleType("antenv.axon_hooks")
        mod._hook = hook

        def set_axon_ntff_profile_hook(h):
            mod._hook = h

        def get_axon_ntff_profile_hook():
            return mod._hook

        mod.set_axon_ntff_profile_hook = set_axon_ntff_profile_hook
        mod.get_axon_ntff_profile_hook = get_axon_ntff_profile_hook
        sys.modules["antenv.axon_hooks"] = mod
        return True
    except Exception:
        return False


_CACHE = {}


def _get_program():
    if "nc" not in _CACHE:
        _CACHE["nc"] = build_program()
    return _CACHE["nc"]


def _shard_inputs(x, qweight, scales, bias):
    try:
        from ml_dtypes import bfloat16
    except ImportError:
        import jax.numpy as jnp

        bfloat16 = np.dtype(jnp.bfloat16)

    # host-side input marshaling (part of the sharding step): cast the
    # activations to bf16 (the matmul input precision) and lay each core's
    # shards out k-major ([K, M] / [K, N] / [KT, N]) so the device streams
    # them with plain contiguous-row DMAs -- no on-device transposes.
    x2 = np.asarray(x, dtype=np.float32).reshape(B * S, K_FULL)
    xbf_t = np.ascontiguousarray(x2.astype(bfloat16).T)  # [K, B*S]
    qweight = np.asarray(qweight, dtype=np.int32)
    scales = np.asarray(scales, dtype=np.float32)
    bias = np.asarray(bias, dtype=np.float32)
    qT = np.ascontiguousarray(qweight.T)                   # [K, NF]
    sT = np.ascontiguousarray(scales.astype(bfloat16).T)   # [KT, NF]
    in_maps = []
    for c in range(N_CORES):
        mb, nb = divmod(c, NB_SHARDS)
        in_maps.append(
            {
                "x": np.ascontiguousarray(
                    xbf_t[:, mb * M_CORE : (mb + 1) * M_CORE]
                ),
                "qweight": np.ascontiguousarray(
                    qT[:, nb * N_CORE : (nb + 1) * N_CORE]
                ),
                "scales": np.ascontiguousarray(
                    sT[:, nb * N_CORE : (nb + 1) * N_CORE]
                ),
                "bias": np.ascontiguousarray(bias[nb * N_CORE : (nb + 1) * N_CORE]),
            }
        )
    return in_maps


def _gather_output(results):
    out = np.empty((B * S, NF), dtype=np.float32)
    for c in range(N_CORES):
        mb, nb = divmod(c, NB_SHARDS)
        out[mb * M_CORE : (mb + 1) * M_CORE, nb * N_CORE : (nb + 1) * N_CORE] = (
            results[c]["out"]
        )
    return out.reshape(B, S, NF)


def run_sharded(x, qweight, scales, bias, **spmd_kwargs):
    """Run on all 8 cores; returns (full_output, BassKernelResults)."""
    if spmd_kwargs.get("trace"):
        enable_ntff_profiling()
    nc = _get_program()
    in_maps = _shard_inputs(x, qweight, scales, bias)
    res = bass_utils.run_bass_kernel_spmd(
        nc, in_maps, core_ids=list(range(N_CORES)), **spmd_kwargs
    )
    return _gather_output(res.results), res


def kernel(x, qweight, scales, bias):
    out, _ = run_sharded(x, qweight, scales, bias)
    return out
